# revision 27
# baseline (speedup 1.0000x reference)
"""Masked L1 loss (per-(b,c) normalized) on 8 Trainium2 NeuronCores.

Layout: batch-dim data parallel. Core i takes batches [2i, 2i+2) of the
[16, 64, 128, 128] inputs -> a [128, 16384] shard (partition = (b, c) pair,
free = h*w). The kernel is DMA-bound, so everything is built around the DMA
stream:

  - pre and gt are repacked on the host into one tile-major tensor
    (per tile: [pre_i | gt_i] blocks), so each tile needs ONE pre/gt DMA
    plus one mask DMA -> 19 input DMAs. All input DMAs use the gpsimd SWDGE
    path (the only one that can downcast in flight); fewer DMAs matter
    because SWDGE descriptor generation is serial (~1 us per DMA).
  - all inputs are DMA'd with an inline f32 -> bf16 cast: HBM reads are
    unchanged but the SBUF-side stream halves and all on-chip operands are
    16-bit, which doubles DVE tensor_tensor throughput (2x mode). Loss
    tolerance is 2e-2; bf16 + fp32 accumulation gives ~1e-4.
  - per tile, DVE runs just two 2x-mode tensor_tensor ops (sd = pre - gt,
    y = sd * mask); the l1 partial sum(|y|) = sum(|d|*mask) comes from ACT
    activation(Abs, accum_out) consuming y. ACT is a pure sink - nothing
    downstream waits on it mid-stream - so the cross-engine hop never
    stalls the pipeline. The last two (small) tiles instead use the all-DVE
    tensor_reduce(add, apply_absolute_value) so the post-last-byte chain
    stays on one engine.
  - the bf16 mask stays resident in SBUF (32 KiB/partition) and is fully
    fetched by pg tile 1, in four 4096-column chunks. Counts (sum of 0/1
    mask, exact in bf16) run early over those chunks, alternating DVE
    tensor_reduce / ACT activation(Copy, accum_out) - both engines stay
    under the 35 us DMA roofline and no count gates the tail.
  - tile sizes shrink geometrically (4096 -> 256), so the final DMA gates
    only the tiny DVE chain and the single output DMA (all partials are in
    one fp32 tile).
Host: loss = sum(l1 / max(ct, 1)) / B.
"""

import sys

if "/opt/trn_rl_repo" not in sys.path:
    sys.path.insert(0, "/opt/trn_rl_repo")

import numpy as np

B, C, H, W = 16, 64, 128, 128
N_CORES = 8
BPC = B // N_CORES          # batches per core = 2
P = BPC * C                 # partitions per core = 128 (one (b,c) pair each)
HW = H * W                  # 16384 free elements per partition

SIZES = [4096, 4096, 2048, 2048, 1536, 1024, 768, 512, 256]   # sum = 16384
NT = len(SIZES)
OFFS = [sum(SIZES[:i]) for i in range(NT)]
N_DVE_TAIL = 2              # last tiles reduce on DVE (no ACT in the tail)

# mask chunk DMAs (lo, hi, after_pg_tile): the whole mask lands by pg1 so
# counts and the mult deps never gate the tail
MASK_CHUNKS = [
    (0, 4096, 0),
    (4096, 8192, 0),
    (8192, 12288, 1),
    (12288, 16384, 1),
]

# count chunks (lo, hi, engine, emit_after_tile); ranges must be fully
# DMA'd by their emit point
COUNT_CHUNKS = [
    (0, 4096, "act", 0),
    (4096, 8192, "dve", 1),
    (8192, 12288, "act", 2),
    (12288, 16384, "dve", 3),
]
NCC = len(COUNT_CHUNKS)

_CACHE = {}


def _build():
    key = "nc"
    if key in _CACHE:
        return _CACHE[key]

    import concourse.bacc as bacc
    import concourse.mybir as mybir
    from concourse.tile import TileContext

    f32 = mybir.dt.float32
    bf16 = mybir.dt.bfloat16
    Alu = mybir.AluOpType
    Act = mybir.ActivationFunctionType

    nc = bacc.Bacc(
        "TRN2",
        target_bir_lowering=False,
        debug=False,
        enable_asserts=False,
        num_devices=N_CORES,
    )

    pgin = nc.dram_tensor("pgin", [P, 2 * HW], f32, kind="ExternalInput").ap()
    mask = nc.dram_tensor("mask", [P, HW], f32, kind="ExternalInput").ap()
    out = nc.dram_tensor("out", [P, NT + NCC], f32, kind="ExternalOutput").ap()

    with TileContext(nc) as tc:
        with (
            tc.tile_pool(name="pg", bufs=4) as pg,
            tc.tile_pool(name="mp", bufs=1) as mp,
            tc.tile_pool(name="work", bufs=4) as work,
            tc.tile_pool(name="acc", bufs=1) as accp,
        ):
            acc = accp.tile([P, NT + NCC], f32, tag="acc")
            l1p = acc[:, 0:NT]
            ctp = acc[:, NT : NT + NCC]
            trash = accp.tile([P, 4096], bf16, tag="trash")
            tmr = mp.tile([P, HW], bf16, tag="mask")   # resident bf16 mask

            def emit_counts(after_tile):
                for ci, (lo, hi, eng, ready) in enumerate(COUNT_CHUNKS):
                    if ready != after_tile:
                        continue
                    if eng == "dve":
                        nc.vector.tensor_reduce(
                            out=ctp[:, ci : ci + 1],
                            in_=tmr[:, lo:hi],
                            axis=mybir.AxisListType.X,
                            op=Alu.add,
                        )
                    else:
                        nc.scalar.activation(
                            out=trash[:, : hi - lo],
                            in_=tmr[:, lo:hi],
                            func=Act.Copy,
                            accum_out=ctp[:, ci : ci + 1],
                        )

            for i in range(NT):
                s, o = SIZES[i], OFFS[i]

                # DMA order: pg_i, then any mask chunks scheduled after it
                # (all mask bytes land by pg1); the final DMA is the last
                # (tiny) pg tile, gating only the short DVE chain
                xt = pg.tile([P, 2 * s], bf16, tag="pg", name=f"xt{i}")
                nc.gpsimd.dma_start(out=xt, in_=pgin[:, 2 * o : 2 * o + 2 * s])
                for lo, hi, after in MASK_CHUNKS:
                    if after == i:
                        nc.gpsimd.dma_start(out=tmr[:, lo:hi], in_=mask[:, lo:hi])

                emit_counts(i)

                # DVE: two 2x-mode TTs; ACT (or DVE for tail tiles) reduces
                sd = work.tile([P, s], bf16, tag="sd")
                y = work.tile([P, s], bf16, tag="y", name=f"y{i}")
                nc.vector.tensor_tensor(
                    out=sd, in0=xt[:, 0:s], in1=xt[:, s : 2 * s], op=Alu.subtract
                )
                nc.vector.tensor_tensor(
                    out=y, in0=sd, in1=tmr[:, o : o + s], op=Alu.mult
                )
                if i < NT - N_DVE_TAIL:
                    nc.scalar.activation(
                        out=trash[:, :s],
                        in_=y,
                        func=Act.Abs,
                        accum_out=l1p[:, i : i + 1],
                    )
                else:
                    nc.vector.tensor_reduce(
                        out=l1p[:, i : i + 1],
                        in_=y,
                        axis=mybir.AxisListType.X,
                        op=Alu.add,
                        apply_absolute_value=True,
                    )

            nc.sync.dma_start(out=out, in_=acc)

    nc.compile()
    _CACHE[key] = nc
    return nc


def _shard(pre, gt, mask):
    in_maps = []
    for i in range(N_CORES):
        sl = slice(i * BPC, (i + 1) * BPC)
        p = np.ascontiguousarray(pre[sl], dtype=np.float32).reshape(P, HW)
        g = np.ascontiguousarray(gt[sl], dtype=np.float32).reshape(P, HW)
        pgin = np.empty((P, 2 * HW), dtype=np.float32)
        for s, o in zip(SIZES, OFFS):
            pgin[:, 2 * o : 2 * o + s] = p[:, o : o + s]
            pgin[:, 2 * o + s : 2 * o + 2 * s] = g[:, o : o + s]
        in_maps.append(
            {
                "pgin": pgin,
                "mask": np.ascontiguousarray(mask[sl], dtype=np.float32).reshape(P, HW),
            }
        )
    return in_maps


def _combine(results, batch_size):
    total = np.float32(0.0)
    for r in results:
        o = np.asarray(r["out"], dtype=np.float32)
        l1 = o[:, :NT].sum(axis=1, dtype=np.float32)
        ct = o[:, NT:].sum(axis=1, dtype=np.float32)
        total += (l1 / np.maximum(ct, np.float32(1.0))).sum(dtype=np.float32)
    return np.asarray(total / np.float32(int(batch_size)), dtype=np.float32)


def run(pre, gt, mask, batch_size, trace=False, **bass_kwargs):
    from concourse.bass_utils import run_bass_kernel_spmd

    nc = _build()
    in_maps = _shard(np.asarray(pre), np.asarray(gt), np.asarray(mask))
    res = run_bass_kernel_spmd(
        nc, in_maps, list(range(N_CORES)), trace=trace, **bass_kwargs
    )
    loss = _combine(res.results, batch_size)
    return loss, res


def kernel(pre, gt, mask, batch_size):
    loss, _ = run(pre, gt, mask, batch_size)
    return loss


# revision 28
# speedup vs baseline: 1.0043x; 1.0043x over previous
"""Masked L1 loss (per-(b,c) normalized) on 8 Trainium2 NeuronCores.

Layout: batch-dim data parallel. Core i takes batches [2i, 2i+2) of the
[16, 64, 128, 128] inputs -> a [128, 16384] shard (partition = (b, c) pair,
free = h*w). The kernel is DMA-bound, so everything is built around the DMA
stream:

  - pre and gt are repacked on the host into one tile-major tensor
    (per tile: [pre_i | gt_i] blocks), so each tile needs ONE pre/gt DMA
    plus one mask DMA -> 19 input DMAs. All input DMAs use the gpsimd SWDGE
    path (the only one that can downcast in flight); fewer DMAs matter
    because SWDGE descriptor generation is serial (~1 us per DMA).
  - all inputs are DMA'd with an inline f32 -> bf16 cast: HBM reads are
    unchanged but the SBUF-side stream halves and all on-chip operands are
    16-bit, which doubles DVE tensor_tensor throughput (2x mode). Loss
    tolerance is 2e-2; bf16 + fp32 accumulation gives ~1e-4.
  - per tile, DVE runs just two 2x-mode tensor_tensor ops (sd = pre - gt,
    y = sd * mask); the l1 partial sum(|y|) = sum(|d|*mask) comes from ACT
    activation(Abs, accum_out) consuming y. ACT is a pure sink - nothing
    downstream waits on it mid-stream - so the cross-engine hop never
    stalls the pipeline. The last two (small) tiles instead use the all-DVE
    tensor_reduce(add, apply_absolute_value) so the post-last-byte chain
    stays on one engine.
  - the bf16 mask stays resident in SBUF (32 KiB/partition) and is fully
    fetched by pg tile 1, in four 4096-column chunks. Counts (sum of 0/1
    mask, exact in bf16) run early over those chunks, alternating DVE
    tensor_reduce / ACT activation(Copy, accum_out) - both engines stay
    under the 35 us DMA roofline and no count gates the tail.
  - tile sizes shrink geometrically (4096 -> 256), so the final DMA gates
    only the tiny DVE chain and the single output DMA (all partials are in
    one fp32 tile).
Host: loss = sum(l1 / max(ct, 1)) / B.
"""

import sys

if "/opt/trn_rl_repo" not in sys.path:
    sys.path.insert(0, "/opt/trn_rl_repo")

import numpy as np

B, C, H, W = 16, 64, 128, 128
N_CORES = 8
BPC = B // N_CORES          # batches per core = 2
P = BPC * C                 # partitions per core = 128 (one (b,c) pair each)
HW = H * W                  # 16384 free elements per partition

SIZES = [4096, 4096, 2048, 2048, 1536, 1024, 768, 512, 256]   # sum = 16384
NT = len(SIZES)
OFFS = [sum(SIZES[:i]) for i in range(NT)]
N_DVE_TAIL = 2              # last tiles reduce on DVE (no ACT in the tail)
ACCUM_TILES = {2, 3, 4}     # subtract fused into the DMA (CCE adder) for
                            # these tiles; must be <= 2048 cols (CCE element
                            # limit per descriptor)

# mask chunk DMAs (lo, hi, after_pg_tile): the whole mask lands by pg1 so
# counts and the mult deps never gate the tail
MASK_CHUNKS = [
    (0, 4096, 0),
    (4096, 8192, 0),
    (8192, 12288, 1),
    (12288, 16384, 1),
]

# count chunks (lo, hi, engine, emit_after_tile); ranges must be fully
# DMA'd by their emit point
COUNT_CHUNKS = [
    (0, 4096, "act", 0),
    (4096, 8192, "dve", 1),
    (8192, 12288, "act", 2),
    (12288, 16384, "dve", 3),
]
NCC = len(COUNT_CHUNKS)

_CACHE = {}


def _build():
    key = "nc"
    if key in _CACHE:
        return _CACHE[key]

    import concourse.bacc as bacc
    import concourse.mybir as mybir
    from concourse.tile import TileContext

    f32 = mybir.dt.float32
    bf16 = mybir.dt.bfloat16
    Alu = mybir.AluOpType
    Act = mybir.ActivationFunctionType

    nc = bacc.Bacc(
        "TRN2",
        target_bir_lowering=False,
        debug=False,
        enable_asserts=False,
        num_devices=N_CORES,
    )

    pgin = nc.dram_tensor("pgin", [P, 2 * HW], f32, kind="ExternalInput").ap()
    mask = nc.dram_tensor("mask", [P, HW], f32, kind="ExternalInput").ap()
    out = nc.dram_tensor("out", [P, NT + NCC], f32, kind="ExternalOutput").ap()

    with TileContext(nc) as tc:
        with (
            tc.tile_pool(name="pg", bufs=4) as pg,
            tc.tile_pool(name="mp", bufs=1) as mp,
            tc.tile_pool(name="work", bufs=4) as work,
            tc.tile_pool(name="acc", bufs=1) as accp,
        ):
            acc = accp.tile([P, NT + NCC], f32, tag="acc")
            l1p = acc[:, 0:NT]
            ctp = acc[:, NT : NT + NCC]
            trash = accp.tile([P, 4096], bf16, tag="trash")
            tmr = mp.tile([P, HW], bf16, tag="mask")   # resident bf16 mask

            def emit_counts(after_tile):
                for ci, (lo, hi, eng, ready) in enumerate(COUNT_CHUNKS):
                    if ready != after_tile:
                        continue
                    if eng == "dve":
                        nc.vector.tensor_reduce(
                            out=ctp[:, ci : ci + 1],
                            in_=tmr[:, lo:hi],
                            axis=mybir.AxisListType.X,
                            op=Alu.add,
                        )
                    else:
                        nc.scalar.activation(
                            out=trash[:, : hi - lo],
                            in_=tmr[:, lo:hi],
                            func=Act.Copy,
                            accum_out=ctp[:, ci : ci + 1],
                        )

            for i in range(NT):
                s, o = SIZES[i], OFFS[i]
                sd = work.tile([P, s], bf16, tag="sd", name=f"sd{i}")

                # DMA order: pg_i (one packed DMA, or a CCE pair computing
                # sd = bf16(pre) + bf16(ng) in the DMA engine for mid-size
                # tiles), then any mask chunks scheduled after it. The final
                # DMA is the last (tiny) pg tile, gating one short DVE chain.
                if i in ACCUM_TILES:
                    nc.gpsimd.dma_start(out=sd, in_=pgin[:, 2 * o : 2 * o + s])
                    nc.gpsimd.dma_start(
                        out=sd,
                        in_=pgin[:, 2 * o + s : 2 * o + 2 * s],
                        accum_op=Alu.add,
                    )
                else:
                    xt = pg.tile([P, 2 * s], bf16, tag="pg", name=f"xt{i}")
                    nc.gpsimd.dma_start(out=xt, in_=pgin[:, 2 * o : 2 * o + 2 * s])
                for lo, hi, after in MASK_CHUNKS:
                    if after == i:
                        nc.gpsimd.dma_start(out=tmr[:, lo:hi], in_=mask[:, lo:hi])

                emit_counts(i)

                # DVE 2x-mode TTs; ACT (or DVE for tail tiles) reduces.
                # ng = -gt on the host, so the combine op is an add.
                y = work.tile([P, s], bf16, tag="y", name=f"y{i}")
                if i not in ACCUM_TILES:
                    nc.vector.tensor_tensor(
                        out=sd, in0=xt[:, 0:s], in1=xt[:, s : 2 * s], op=Alu.add
                    )
                nc.vector.tensor_tensor(
                    out=y, in0=sd, in1=tmr[:, o : o + s], op=Alu.mult
                )
                if i < NT - N_DVE_TAIL:
                    nc.scalar.activation(
                        out=trash[:, :s],
                        in_=y,
                        func=Act.Abs,
                        accum_out=l1p[:, i : i + 1],
                    )
                else:
                    nc.vector.tensor_reduce(
                        out=l1p[:, i : i + 1],
                        in_=y,
                        axis=mybir.AxisListType.X,
                        op=Alu.add,
                        apply_absolute_value=True,
                    )

            nc.sync.dma_start(out=out, in_=acc)

    nc.compile()
    _CACHE[key] = nc
    return nc


def _shard(pre, gt, mask):
    in_maps = []
    for i in range(N_CORES):
        sl = slice(i * BPC, (i + 1) * BPC)
        p = np.ascontiguousarray(pre[sl], dtype=np.float32).reshape(P, HW)
        g = np.ascontiguousarray(gt[sl], dtype=np.float32).reshape(P, HW)
        pgin = np.empty((P, 2 * HW), dtype=np.float32)
        for s, o in zip(SIZES, OFFS):
            pgin[:, 2 * o : 2 * o + s] = p[:, o : o + s]
            pgin[:, 2 * o + s : 2 * o + 2 * s] = -g[:, o : o + s]
        in_maps.append(
            {
                "pgin": pgin,
                "mask": np.ascontiguousarray(mask[sl], dtype=np.float32).reshape(P, HW),
            }
        )
    return in_maps


def _combine(results, batch_size):
    total = np.float32(0.0)
    for r in results:
        o = np.asarray(r["out"], dtype=np.float32)
        l1 = o[:, :NT].sum(axis=1, dtype=np.float32)
        ct = o[:, NT:].sum(axis=1, dtype=np.float32)
        total += (l1 / np.maximum(ct, np.float32(1.0))).sum(dtype=np.float32)
    return np.asarray(total / np.float32(int(batch_size)), dtype=np.float32)


def run(pre, gt, mask, batch_size, trace=False, **bass_kwargs):
    from concourse.bass_utils import run_bass_kernel_spmd

    nc = _build()
    in_maps = _shard(np.asarray(pre), np.asarray(gt), np.asarray(mask))
    res = run_bass_kernel_spmd(
        nc, in_maps, list(range(N_CORES)), trace=trace, **bass_kwargs
    )
    loss = _combine(res.results, batch_size)
    return loss, res


def kernel(pre, gt, mask, batch_size):
    loss, _ = run(pre, gt, mask, batch_size)
    return loss


# revision 29
# speedup vs baseline: 1.0057x; 1.0014x over previous
"""Masked L1 loss (per-(b,c) normalized) on 8 Trainium2 NeuronCores.

Layout: batch-dim data parallel. Core i takes batches [2i, 2i+2) of the
[16, 64, 128, 128] inputs -> a [128, 16384] shard (partition = (b, c) pair,
free = h*w). The kernel is DMA-bound, so everything is built around the DMA
stream:

  - pre and gt are repacked on the host into one tile-major tensor
    (per tile: [pre_i | gt_i] blocks), so each tile needs ONE pre/gt DMA
    plus one mask DMA -> 19 input DMAs. All input DMAs use the gpsimd SWDGE
    path (the only one that can downcast in flight); fewer DMAs matter
    because SWDGE descriptor generation is serial (~1 us per DMA).
  - all inputs are DMA'd with an inline f32 -> bf16 cast: HBM reads are
    unchanged but the SBUF-side stream halves and all on-chip operands are
    16-bit, which doubles DVE tensor_tensor throughput (2x mode). Loss
    tolerance is 2e-2; bf16 + fp32 accumulation gives ~1e-4.
  - per tile, DVE runs just two 2x-mode tensor_tensor ops (sd = pre - gt,
    y = sd * mask); the l1 partial sum(|y|) = sum(|d|*mask) comes from ACT
    activation(Abs, accum_out) consuming y. ACT is a pure sink - nothing
    downstream waits on it mid-stream - so the cross-engine hop never
    stalls the pipeline. The last two (small) tiles instead use the all-DVE
    tensor_reduce(add, apply_absolute_value) so the post-last-byte chain
    stays on one engine.
  - the bf16 mask stays resident in SBUF (32 KiB/partition) and is fully
    fetched by pg tile 1, in four 4096-column chunks. Counts (sum of 0/1
    mask, exact in bf16) run early over those chunks, alternating DVE
    tensor_reduce / ACT activation(Copy, accum_out) - both engines stay
    under the 35 us DMA roofline and no count gates the tail.
  - tile sizes shrink geometrically (4096 -> 256), so the final DMA gates
    only the tiny DVE chain and the single output DMA (all partials are in
    one fp32 tile).
Host: loss = sum(l1 / max(ct, 1)) / B.
"""

import sys

if "/opt/trn_rl_repo" not in sys.path:
    sys.path.insert(0, "/opt/trn_rl_repo")

import numpy as np

B, C, H, W = 16, 64, 128, 128
N_CORES = 8
BPC = B // N_CORES          # batches per core = 2
P = BPC * C                 # partitions per core = 128 (one (b,c) pair each)
HW = H * W                  # 16384 free elements per partition

SIZES = [4096, 4096, 2048, 2048, 1536, 1024, 768, 512, 256]   # sum = 16384
NT = len(SIZES)
OFFS = [sum(SIZES[:i]) for i in range(NT)]
N_DVE_TAIL = 2              # last tiles reduce on DVE (no ACT in the tail)
ACCUM_TILES = {2, 3, 4}     # subtract fused into the DMA (CCE adder) for
                            # these tiles; must be <= 2048 cols (CCE element
                            # limit per descriptor)
SPLIT_TILES = {0}           # oversize tiles whose DMA is split into two
                            # CCE-safe half pairs (compute tiling unchanged)

# mask chunk DMAs (lo, hi, after_pg_tile): the whole mask lands by pg1 so
# counts and the mult deps never gate the tail
MASK_CHUNKS = [
    (0, 4096, 0),
    (4096, 8192, 0),
    (8192, 12288, 1),
    (12288, 16384, 1),
]

# count chunks (lo, hi, engine, emit_after_tile); ranges must be fully
# DMA'd by their emit point
COUNT_CHUNKS = [
    (0, 4096, "act", 0),
    (4096, 8192, "dve", 1),
    (8192, 12288, "act", 2),
    (12288, 16384, "dve", 3),
]
NCC = len(COUNT_CHUNKS)

_CACHE = {}


def _build():
    key = "nc"
    if key in _CACHE:
        return _CACHE[key]

    import concourse.bacc as bacc
    import concourse.mybir as mybir
    from concourse.tile import TileContext

    f32 = mybir.dt.float32
    bf16 = mybir.dt.bfloat16
    Alu = mybir.AluOpType
    Act = mybir.ActivationFunctionType

    nc = bacc.Bacc(
        "TRN2",
        target_bir_lowering=False,
        debug=False,
        enable_asserts=False,
        num_devices=N_CORES,
    )

    pgin = nc.dram_tensor("pgin", [P, 2 * HW], f32, kind="ExternalInput").ap()
    mask = nc.dram_tensor("mask", [P, HW], f32, kind="ExternalInput").ap()
    out = nc.dram_tensor("out", [P, NT + NCC], f32, kind="ExternalOutput").ap()

    with TileContext(nc) as tc:
        with (
            tc.tile_pool(name="pg", bufs=4) as pg,
            tc.tile_pool(name="mp", bufs=1) as mp,
            tc.tile_pool(name="work", bufs=4) as work,
            tc.tile_pool(name="acc", bufs=1) as accp,
        ):
            acc = accp.tile([P, NT + NCC], f32, tag="acc")
            l1p = acc[:, 0:NT]
            ctp = acc[:, NT : NT + NCC]
            trash = accp.tile([P, 4096], bf16, tag="trash")
            tmr = mp.tile([P, HW], bf16, tag="mask")   # resident bf16 mask

            def emit_counts(after_tile):
                for ci, (lo, hi, eng, ready) in enumerate(COUNT_CHUNKS):
                    if ready != after_tile:
                        continue
                    if eng == "dve":
                        nc.vector.tensor_reduce(
                            out=ctp[:, ci : ci + 1],
                            in_=tmr[:, lo:hi],
                            axis=mybir.AxisListType.X,
                            op=Alu.add,
                        )
                    else:
                        nc.scalar.activation(
                            out=trash[:, : hi - lo],
                            in_=tmr[:, lo:hi],
                            func=Act.Copy,
                            accum_out=ctp[:, ci : ci + 1],
                        )

            for i in range(NT):
                s, o = SIZES[i], OFFS[i]
                sd = work.tile([P, s], bf16, tag="sd", name=f"sd{i}")

                # DMA order: pg_i (one packed DMA, or a CCE pair computing
                # sd = bf16(pre) + bf16(ng) in the DMA engine for mid-size
                # tiles), then any mask chunks scheduled after it. The final
                # DMA is the last (tiny) pg tile, gating one short DVE chain.
                if i in SPLIT_TILES:
                    h = s // 2
                    for k in (0, 1):
                        nc.gpsimd.dma_start(
                            out=sd[:, k * h : (k + 1) * h],
                            in_=pgin[:, 2 * o + k * h : 2 * o + (k + 1) * h],
                        )
                        nc.gpsimd.dma_start(
                            out=sd[:, k * h : (k + 1) * h],
                            in_=pgin[:, 2 * o + s + k * h : 2 * o + s + (k + 1) * h],
                            accum_op=Alu.add,
                        )
                elif i in ACCUM_TILES:
                    nc.gpsimd.dma_start(out=sd, in_=pgin[:, 2 * o : 2 * o + s])
                    nc.gpsimd.dma_start(
                        out=sd,
                        in_=pgin[:, 2 * o + s : 2 * o + 2 * s],
                        accum_op=Alu.add,
                    )
                else:
                    xt = pg.tile([P, 2 * s], bf16, tag="pg", name=f"xt{i}")
                    nc.gpsimd.dma_start(out=xt, in_=pgin[:, 2 * o : 2 * o + 2 * s])
                for lo, hi, after in MASK_CHUNKS:
                    if after == i:
                        nc.gpsimd.dma_start(out=tmr[:, lo:hi], in_=mask[:, lo:hi])

                emit_counts(i)

                # DVE 2x-mode TTs; ACT (or DVE for tail tiles) reduces.
                # ng = -gt on the host, so the combine op is an add.
                y = work.tile([P, s], bf16, tag="y", name=f"y{i}")
                if i not in ACCUM_TILES and i not in SPLIT_TILES:
                    nc.vector.tensor_tensor(
                        out=sd, in0=xt[:, 0:s], in1=xt[:, s : 2 * s], op=Alu.add
                    )
                nc.vector.tensor_tensor(
                    out=y, in0=sd, in1=tmr[:, o : o + s], op=Alu.mult
                )
                if i < NT - N_DVE_TAIL:
                    nc.scalar.activation(
                        out=trash[:, :s],
                        in_=y,
                        func=Act.Abs,
                        accum_out=l1p[:, i : i + 1],
                    )
                else:
                    nc.vector.tensor_reduce(
                        out=l1p[:, i : i + 1],
                        in_=y,
                        axis=mybir.AxisListType.X,
                        op=Alu.add,
                        apply_absolute_value=True,
                    )

            nc.sync.dma_start(out=out, in_=acc)

    nc.compile()
    _CACHE[key] = nc
    return nc


def _shard(pre, gt, mask):
    in_maps = []
    for i in range(N_CORES):
        sl = slice(i * BPC, (i + 1) * BPC)
        p = np.ascontiguousarray(pre[sl], dtype=np.float32).reshape(P, HW)
        g = np.ascontiguousarray(gt[sl], dtype=np.float32).reshape(P, HW)
        pgin = np.empty((P, 2 * HW), dtype=np.float32)
        for s, o in zip(SIZES, OFFS):
            pgin[:, 2 * o : 2 * o + s] = p[:, o : o + s]
            pgin[:, 2 * o + s : 2 * o + 2 * s] = -g[:, o : o + s]
        in_maps.append(
            {
                "pgin": pgin,
                "mask": np.ascontiguousarray(mask[sl], dtype=np.float32).reshape(P, HW),
            }
        )
    return in_maps


def _combine(results, batch_size):
    total = np.float32(0.0)
    for r in results:
        o = np.asarray(r["out"], dtype=np.float32)
        l1 = o[:, :NT].sum(axis=1, dtype=np.float32)
        ct = o[:, NT:].sum(axis=1, dtype=np.float32)
        total += (l1 / np.maximum(ct, np.float32(1.0))).sum(dtype=np.float32)
    return np.asarray(total / np.float32(int(batch_size)), dtype=np.float32)


def run(pre, gt, mask, batch_size, trace=False, **bass_kwargs):
    from concourse.bass_utils import run_bass_kernel_spmd

    nc = _build()
    in_maps = _shard(np.asarray(pre), np.asarray(gt), np.asarray(mask))
    res = run_bass_kernel_spmd(
        nc, in_maps, list(range(N_CORES)), trace=trace, **bass_kwargs
    )
    loss = _combine(res.results, batch_size)
    return loss, res


def kernel(pre, gt, mask, batch_size):
    loss, _ = run(pre, gt, mask, batch_size)
    return loss
